# revision 22
# baseline (speedup 1.0000x reference)
"""BinaryTreeLSTM on 8 TRN2 NeuronCores.

Strategy: tensor-parallel over the 8H gate dimension (sharding hint).
Key algebraic facts exploited:
  - The reference keeps only the first H dims of h_new/c_new per level, so
    only gate rows {q*2H + [0:H]} of the 8H weight rows ever matter
    ("kept gates": 4H instead of 8H -> 2x less matmul work).
  - c_cat[:, :H] is the LEFT child's c only, elementwise per hidden dim ->
    c never needs to be exchanged between cores; only h is all-gathered.
  - At the leaf level h = c = 0 -> the W_hh matmul and the f-gate*c term
    are skipped entirely.
Each core m owns hidden dims [128m, 128m+128) of each of the i,f,g,o gates
(a 512-wide gate slice). Per level it computes gates.T (feature-major:
gate dims on PSUM partitions, nodes on the free axis), applies the LSTM
cell elementwise, and all-gathers its h.T slice (128, n) into the full
h.T (1024, n) for the next level.

Perf notes vs the first working version (432us):
  - emb/W_ih in bf16 (halves startup DMA; same 1 cycle/row PE rate).
  - h is gathered in fp8e4m3 for the three big levels (leaf, k=10, k=9):
    halves AllGather wire bytes where it matters; numpy sim shows final
    rel err 2.8e-3 (vs 2e-2 budget). Mixed-dtype matmul (bf16 stationary
    x fp8 moving) is architecturally allowed and runs at the same rate.
  - A tiny warmup AllGather at t=0 absorbs the ~14us cold-start cost of
    the first real collective.
  - Bias is folded into the XW precompute (x@W_ih.T + b for all interior
    nodes), so per-level activations are biasless and the i,f gates share
    one fused sigmoid; the live x matmuls for the top tree are gone.
  - h-slab loads are split across 4 HW-DGE queues (sync/scalar/vector/
    tensor) instead of one software-DGE gpsimd queue (~1us fixed cost,
    and gpsimd also dispatches the collectives).
  - Leaf h is gathered in 2 AllGathers of 1024 cols (fewer collective
    dispatches; each collective has a ~4.5us floor).
"""

import sys

for p in ("/opt/trn_rl_repo",):
    if p not in sys.path:
        sys.path.insert(0, p)

import numpy as np

import concourse.bass as bass
import concourse.bacc as bacc
import concourse.mybir as mybir
import concourse.tile as tile
from concourse import bass_utils

H = 1024
I = 1024
DEPTH = 12
NCORES = 8
P = 128            # partitions / per-core hidden slice
GS = 4 * P         # per-core gate slice (i,f,g,o each P wide) = 512
NCHUNK = 512       # node-column chunk (PSUM bank = 512 fp32)
F32 = mybir.dt.float32
BF16 = mybir.dt.bfloat16
F8 = mybir.dt.float8e4
AF = mybir.ActivationFunctionType

_CACHE = {}


def _h_dtype(k):
    """dtype of level-k h as gathered/consumed by level k-1."""
    return F8 if k >= 7 else BF16


def _build():
    nc = bacc.Bacc(
        "TRN2",
        target_bir_lowering=False,
        debug=False,
        enable_asserts=False,
        num_devices=NCORES,
    )

    # Host-packed layouts: per-partition-contiguous so each load is 128
    # descriptors of 8-16KB (startup is DMA-descriptor-throughput-bound
    # with feature-major layouts: thousands of 1KB rows).
    # embP chunks 0..3 = xw cols [0:2048), 4..7 = leaf cols [2047:4095).
    embP_d = nc.dram_tensor("embP", (8, P, I // P * NCHUNK), BF16,
                            kind="ExternalInput")
    wihP_d = nc.dram_tensor("wihP", (P, I // P * GS), BF16, kind="ExternalInput")
    whhP_d = nc.dram_tensor("whhP", (P, 2 * H // P * GS), BF16,
                            kind="ExternalInput")
    bias_d = nc.dram_tensor("bias", (P, 4), F32, kind="ExternalInput")
    ident_d = nc.dram_tensor("ident", (P, P), BF16, kind="ExternalInput")
    out_d = nc.dram_tensor("out", (2 * P, 1), F32, kind="ExternalOutput")

    KX = I // P        # 8 contraction chunks for the x part
    KH = 2 * H // P    # 16 contraction chunks for the hh part
    rg = [list(range(NCORES))]

    with tile.TileContext(nc) as tc:
        with (
            tc.tile_pool(name="wpool", bufs=1) as wpool,
            tc.tile_pool(name="xpool", bufs=2) as xpool,
            tc.tile_pool(name="spool", bufs=2) as spool,
            tc.tile_pool(name="state", bufs=2) as state,
            tc.tile_pool(name="ewpool", bufs=2) as ewpool,
            tc.tile_pool(name="psum", bufs=8, space=bass.MemorySpace.PSUM) as psum,
            tc.tile_pool(name="dram", bufs=2, space=bass.MemorySpace.DRAM) as dram,
        ):
            # ---- warmup collective: absorb the cold-start cost ----------
            wz = wpool.tile([P, 1], F32)
            nc.vector.memset(wz[:], 0.0)
            wag_in = dram.tile([P, 1], F32, name="wagin")
            wag_out = dram.tile([NCORES * P, 1], F32, name="wagout",
                                addr_space="Shared")
            nc.sync.dma_start(wag_in[:], wz[:])
            nc.gpsimd.collective_compute(
                "AllGather",
                mybir.AluOpType.bypass,
                replica_groups=rg,
                ins=[wag_in.opt()],
                outs=[wag_out.opt()],
            )

            # ---- resident weights, feature-major ------------------------
            # [:, c, q*128:(q+1)*128] is the stationary (K=128, M=128) tile
            # for contraction chunk c, gate q
            wih = wpool.tile([P, KX, GS], BF16)
            whh = wpool.tile([P, KH, GS], BF16)
            bias = wpool.tile([P, 4], F32)
            ident = wpool.tile([P, P], BF16)
            nc.sync.dma_start(bias[:], bias_d[:])
            nc.sync.dma_start(ident[:], ident_d[:])
            nc.sync.dma_start(
                wih[:], wihP_d[:].rearrange("p (a g) -> p a g", a=KX)
            )
            nc.scalar.dma_start(
                whh[:], whhP_d[:].rearrange("p (c g) -> p c g", c=KH)
            )

            # ---- phase structure ----------------------------------------
            # 1. leaf level (k=11): x-only gates, elementwise, chunked;
            #    AllGathers (fp8, 1024-col chunks) start flowing early.
            # 2. XW precompute: x@W_ih.T + b for ALL interior nodes (heap
            #    rows 0..2046) into SBUF - dense PE work that runs while
            #    the leaf AllGathers drain, and removes the x part (and
            #    the bias) from the recurrent critical path entirely.
            # 3. levels 10..0: hh-only PSUM groups + xw combine + cell.
            xw = wpool.tile([P, 4, 4 * NCHUNK], BF16)  # (128, 4, 2048)

            lvl = {}

            def get_level(k):
                if k not in lvl:
                    n = 2 ** k
                    h_new = state.tile(
                        [P, max(n, 2)], _h_dtype(k), tag="hst", bufs=2,
                        name=f"h{k}"
                    )
                    c_new = state.tile(
                        [P, max(n, 2)], F32, tag="cst", bufs=3, name=f"c{k}"
                    )
                    lvl[k] = {"h": h_new, "c": c_new, "hgat": []}
                return lvl[k]

            def emit_ag(k, p0, pw):
                """AllGather h_new[:, p0:p0+pw] to all cores (via DRAM)."""
                L = lvl[k]
                hdt = _h_dtype(k)
                ag_in = dram.tile([P, pw], hdt, tag="agin", bufs=6,
                                  name=f"agin{k}_{p0}")
                ag_out = dram.tile([NCORES * P, pw], hdt, tag="agout",
                                   bufs=10, name=f"agout{k}_{p0}",
                                   addr_space="Shared")
                nc.sync.dma_start(ag_in[:], L["h"][:, p0:p0 + pw])
                nc.gpsimd.collective_compute(
                    "AllGather",
                    mybir.AluOpType.bypass,
                    replica_groups=rg,
                    ins=[ag_in.opt()],
                    outs=[ag_out.opt()],
                )
                L["hgat"].append((ag_out, pw))

            # ---- phase 1: leaf level ------------------------------------
            K = DEPTH - 1
            nl = 2 ** K
            get_level(K)
            h_leaf, c_leaf = lvl[K]["h"], lvl[K]["c"]
            for j in range(nl // NCHUNK):
                j0 = j * NCHUNK
                ex = xpool.tile([P, KX, NCHUNK], BF16, tag="ex", name=f"exL{j}")
                eng = nc.sync if j % 2 == 0 else nc.scalar
                eng.dma_start(
                    ex[:],
                    embP_d[4 + j].rearrange("p (a w) -> p a w", a=KX),
                )
                ps = [None] * 4
                for q in (0, 2, 3):
                    ps[q] = psum.tile([P, NCHUNK], F32, tag="ps", name=f"psL{j}_{q}")
                for q in (0, 2, 3):
                    for a in range(KX):
                        nc.tensor.matmul(
                            ps[q][:], wih[:, a, q * P:(q + 1) * P], ex[:, a, :],
                            start=(a == 0), stop=(a == KX - 1),
                        )
                # leaf cell: c = sig(i)*tanh(g), h = sig(o)*tanh(c)
                sig_i = ewpool.tile([P, NCHUNK], F32, tag="si")
                tan_g = ewpool.tile([P, NCHUNK], F32, tag="tg")
                sig_o = ewpool.tile([P, NCHUNK], F32, tag="so")
                nc.scalar.activation(sig_i[:], ps[0][:], AF.Sigmoid, bias=bias[:, 0:1])
                nc.scalar.activation(tan_g[:], ps[2][:], AF.Tanh, bias=bias[:, 2:3])
                nc.scalar.activation(sig_o[:], ps[3][:], AF.Sigmoid, bias=bias[:, 3:4])
                nc.vector.tensor_mul(c_leaf[:, j0:j0 + NCHUNK], sig_i[:], tan_g[:])
                tan_c = ewpool.tile([P, NCHUNK], F32, tag="tc")
                nc.scalar.activation(tan_c[:], c_leaf[:, j0:j0 + NCHUNK], AF.Tanh)
                nc.vector.tensor_mul(h_leaf[:, j0:j0 + NCHUNK], sig_o[:], tan_c[:])
                if (j0 + NCHUNK) % 1024 == 0:
                    emit_ag(K, j0 + NCHUNK - 1024, 1024)

            # ---- phase 2: XW precompute for heap rows 0..2047 -----------
            for j in range(4):
                j0 = j * NCHUNK
                ex = xpool.tile([P, KX, NCHUNK], BF16, tag="ex", name=f"exP{j}")
                eng = nc.sync if j % 2 == 0 else nc.scalar
                eng.dma_start(
                    ex[:],
                    embP_d[j].rearrange("p (a w) -> p a w", a=KX),
                )
                for q in range(4):
                    pt = psum.tile([P, NCHUNK], F32, tag="ps", name=f"psP{j}_{q}")
                    for a in range(KX):
                        nc.tensor.matmul(
                            pt[:], wih[:, a, q * P:(q + 1) * P], ex[:, a, :],
                            start=(a == 0), stop=(a == KX - 1),
                        )
                    # fold the bias in here: downstream gates are biasless
                    nc.scalar.activation(
                        xw[:, q, j0:j0 + NCHUNK], pt[:], AF.Identity,
                        bias=bias[:, q:q + 1]
                    )

            # ---- phase 3: recurrent sweep, hh only ----------------------
            slab_engines = [nc.sync, nc.scalar]
            for k in range(DEPTH - 2, -1, -1):
                n = 2 ** k
                base = n - 1
                get_level(k)
                h_new, c_new = lvl[k]["h"], lvl[k]["c"]
                c_prev = lvl[k + 1]["c"]
                hgat = lvl[k + 1]["hgat"]
                sdt = _h_dtype(k + 1)
                # NB: chunking k=9 at 256 was tried and LOST ~5us: the
                # matmul phase is LDW/dispatch-bound (~0.26us per matmul
                # regardless of rows<=512), so halving rows doubles LDW.
                cw = NCHUNK if k >= 9 else max(n, 1)
                for j in range((n + cw - 1) // cw):
                    j0 = j * cw
                    w = min(cw, n - j0)
                    wp = max(w, 2)

                    # gathered h.T slab: [p, cb, col] = h[cb*128+p, col];
                    # cols = level-(k+1) node index (2 children per node).
                    # 2-way split across the HW-DGE queues (sync/scalar),
                    # 4 cb-blocks each; avoids gpsimd's ~1us SWDGE overhead
                    # and keeps gpsimd free for collective dispatch.
                    slab = spool.tile([P, KX, 2 * wp], sdt, tag="slab",
                                      name=f"sl{k}_{j}")
                    pw = hgat[0][1]
                    nsplit = len(slab_engines)
                    cbs = KX // nsplit  # cb-blocks per split
                    for i, eng in enumerate(slab_engines):
                        r0, r1 = i * cbs * P, (i + 1) * cbs * P
                        pos, off, need = 2 * j0, 0, 2 * w
                        while need > 0:
                            pj, pc = divmod(pos, pw)
                            take = min(need, pw - pc)
                            eng.dma_start(
                                slab[:, i * cbs:(i + 1) * cbs, off:off + take],
                                hgat[pj][0][r0:r1, pc:pc + take].rearrange(
                                    "(c p) w -> p c w", p=P
                                ),
                            )
                            pos += take; off += take; need -= take
                        if wp != w:
                            eng.dma_start(
                                slab[:, i * cbs:(i + 1) * cbs, 2 * w:4 * w],
                                hgat[0][0][r0:r1, 0:2 * w].rearrange(
                                    "(c p) w -> p c w", p=P
                                ),
                            )

                    fused = 4 * wp <= NCHUNK  # all 4 gates in one PSUM bank
                    if fused:
                        ps4 = psum.tile([P, 4, wp], F32, tag="ps",
                                        name=f"ps{k}_{j}")
                        outs = [ps4[:, q] for q in range(4)]
                    else:
                        tiles = [psum.tile([P, wp], F32, tag="ps",
                                           name=f"ps{k}_{j}_{q}") for q in range(4)]
                        outs = [t[:] for t in tiles]
                    # gates = xw (identity matmul seeds psum with the
                    # precomputed x part + bias) + hh
                    for q in range(4):
                        nc.tensor.matmul(
                            outs[q], ident[:],
                            xw[:, q, base + j0: base + j0 + wp],
                            start=True, stop=False,
                        )
                        for c in range(KH):
                            nc.tensor.matmul(
                                outs[q],
                                whh[:, c, q * P:(q + 1) * P],
                                slab[:, c % KX, (c // KX)::2],
                                start=False, stop=(c == KH - 1),
                            )

                    # LSTM cell (biasless): i,f share one fused sigmoid
                    sif = ewpool.tile([P, 2, wp], F32, tag="sif")
                    tg = ewpool.tile([P, wp], F32, tag="tg2")
                    so = ewpool.tile([P, wp], F32, tag="so2")
                    if fused:
                        nc.scalar.activation(sif[:], ps4[:, 0:2], AF.Sigmoid)
                    else:
                        nc.scalar.activation(sif[:, 0], outs[0], AF.Sigmoid)
                        nc.scalar.activation(sif[:, 1], outs[1], AF.Sigmoid)
                    nc.scalar.activation(tg[:], outs[2], AF.Tanh)
                    nc.scalar.activation(so[:], outs[3], AF.Sigmoid)

                    t2 = ewpool.tile([P, wp], F32, tag="t2")
                    nc.vector.tensor_mul(t2[:], sif[:, 0], tg[:])
                    if wp == w:
                        c_left = c_prev[:, 2 * j0: 2 * j0 + 2 * w: 2]
                    else:
                        c_left = c_prev[:, 0:2]
                    t1 = ewpool.tile([P, wp], F32, tag="t1")
                    nc.vector.tensor_mul(t1[:], sif[:, 1], c_left)
                    nc.vector.tensor_add(c_new[:, j0:j0 + wp], t1[:], t2[:])

                    tan_c = ewpool.tile([P, wp], F32, tag="tc")
                    nc.scalar.activation(tan_c[:], c_new[:, j0:j0 + wp], AF.Tanh)
                    if k > 0:
                        nc.vector.tensor_mul(h_new[:, j0:j0 + wp], so[:], tan_c[:])
                        emit_ag(k, j0, w)
                    else:
                        h_root = ewpool.tile([P, 2], F32, tag="hroot")
                        nc.vector.tensor_mul(h_root[:], so[:], tan_c[:])
                        nc.sync.dma_start(out_d[0:P, :], h_root[:, 0:1])
                        nc.sync.dma_start(out_d[P:2 * P, :], c_new[:, 0:1])

    nc.compile()
    return nc


def _prep_inputs(emb, W_ih, W_hh, b_ih, b_hh):
    """Host-side sharding: kept-gate rows, per-core slices, transposes."""
    import ml_dtypes

    emb = np.asarray(emb, dtype=np.float32)
    W_ih = np.asarray(W_ih, dtype=np.float32)
    W_hh = np.asarray(W_hh, dtype=np.float32)
    b = np.asarray(b_ih, dtype=np.float32) + np.asarray(b_hh, dtype=np.float32)

    embT = emb.T.astype(ml_dtypes.bfloat16)  # (I, 4095)
    # pack per-partition-contiguous: embP[ci, p, a*512+w] = embT[a*128+p, c0+w]
    embP = np.empty((8, P, 8 * NCHUNK), ml_dtypes.bfloat16)
    nl1 = 2 ** (DEPTH - 1) - 1  # 2047
    for ci in range(8):
        c0 = ci * NCHUNK if ci < 4 else nl1 + (ci - 4) * NCHUNK
        blk = embT[:, c0:c0 + NCHUNK].reshape(8, P, NCHUNK)  # (a, p, w)
        embP[ci] = blk.transpose(1, 0, 2).reshape(P, 8 * NCHUNK)
    in_maps = []
    for m in range(NCORES):
        rows = np.concatenate(
            [np.arange(q * 2 * H + m * P, q * 2 * H + m * P + P) for q in range(4)]
        )
        wihT = W_ih[rows, :].T.astype(ml_dtypes.bfloat16)   # (1024, 512)
        whhT = W_hh[rows, :].T.astype(ml_dtypes.bfloat16)   # (2048, 512)
        wihP = np.ascontiguousarray(
            wihT.reshape(8, P, GS).transpose(1, 0, 2).reshape(P, 8 * GS)
        )
        whhP = np.ascontiguousarray(
            whhT.reshape(16, P, GS).transpose(1, 0, 2).reshape(P, 16 * GS)
        )
        bias = np.ascontiguousarray(b[rows].reshape(4, P).T)  # (128, 4)
        ident = np.eye(P, dtype=np.float32).astype(ml_dtypes.bfloat16)
        in_maps.append({"embP": embP, "wihP": wihP, "whhP": whhP,
                        "bias": bias, "ident": ident})
    return in_maps


def _install_profile_hook():
    """The agent image's antenv lacks axon_hooks; synthesize it so
    run_bass_kernel_spmd(trace=True) can capture NTFF profiles."""
    import types

    if "antenv.axon_hooks" in sys.modules:
        return
    try:
        from trn_agent_boot.trn_boot import _ntff_profile_via_ctypes
    except ImportError:
        return
    hook = _ntff_profile_via_ctypes("/opt/axon/libaxon_pjrt.so")
    mod = types.ModuleType("antenv.axon_hooks")
    mod._hook = hook
    mod.set_axon_ntff_profile_hook = lambda h: setattr(mod, "_hook", h)
    mod.get_axon_ntff_profile_hook = lambda: mod._hook
    sys.modules["antenv.axon_hooks"] = mod
    import antenv

    antenv.axon_hooks = mod


def _run(in_maps, trace=False):
    if trace:
        _install_profile_hook()
    if "nc" not in _CACHE:
        _CACHE["nc"] = _build()
    nc = _CACHE["nc"]
    res = bass_utils.run_bass_kernel_spmd(
        nc, in_maps, core_ids=list(range(NCORES)), trace=trace
    )
    return res


def _assemble(results):
    out = np.zeros((1, 2 * H), dtype=np.float32)
    for m in range(NCORES):
        o = results[m]["out"].reshape(2 * P)
        out[0, m * P:(m + 1) * P] = o[0:P]
        out[0, H + m * P: H + (m + 1) * P] = o[P:2 * P]
    return out


def kernel(emb, W_ih, W_hh, b_ih, b_hh):
    in_maps = _prep_inputs(emb, W_ih, W_hh, b_ih, b_hh)
    res = _run(in_maps, trace=False)
    return _assemble(res.results)


# revision 27
# speedup vs baseline: 1.0224x; 1.0224x over previous
"""BinaryTreeLSTM on 8 TRN2 NeuronCores.

Strategy: tensor-parallel over the 8H gate dimension (sharding hint).
Key algebraic facts exploited:
  - The reference keeps only the first H dims of h_new/c_new per level, so
    only gate rows {q*2H + [0:H]} of the 8H weight rows ever matter
    ("kept gates": 4H instead of 8H -> 2x less matmul work).
  - c_cat[:, :H] is the LEFT child's c only, elementwise per hidden dim ->
    c never needs to be exchanged between cores; only h is all-gathered.
  - At the leaf level h = c = 0 -> the W_hh matmul and the f-gate*c term
    are skipped entirely.
Each core m owns hidden dims [128m, 128m+128) of each of the i,f,g,o gates
(a 512-wide gate slice). Per level it computes gates.T (feature-major:
gate dims on PSUM partitions, nodes on the free axis), applies the LSTM
cell elementwise, and all-gathers its h.T slice (128, n) into the full
h.T (1024, n) for the next level.

Perf notes vs the first working version (432us):
  - emb/W_ih in bf16 (halves startup DMA; same 1 cycle/row PE rate).
  - h is gathered in fp8e4m3 for the three big levels (leaf, k=10, k=9):
    halves AllGather wire bytes where it matters; numpy sim shows final
    rel err 2.8e-3 (vs 2e-2 budget). Mixed-dtype matmul (bf16 stationary
    x fp8 moving) is architecturally allowed and runs at the same rate.
  - A tiny warmup AllGather at t=0 absorbs the ~14us cold-start cost of
    the first real collective.
  - Bias is folded into the XW precompute (x@W_ih.T + b for all interior
    nodes), so per-level activations are biasless and the i,f gates share
    one fused sigmoid; the live x matmuls for the top tree are gone.
  - h-slab loads are split across 4 HW-DGE queues (sync/scalar/vector/
    tensor) instead of one software-DGE gpsimd queue (~1us fixed cost,
    and gpsimd also dispatches the collectives).
  - Leaf h is gathered in 2 AllGathers of 1024 cols (fewer collective
    dispatches; each collective has a ~4.5us floor).
"""

import sys

for p in ("/opt/trn_rl_repo",):
    if p not in sys.path:
        sys.path.insert(0, p)

import numpy as np

import concourse.bass as bass
import concourse.bacc as bacc
import concourse.mybir as mybir
import concourse.tile as tile
from concourse import bass_utils

H = 1024
I = 1024
DEPTH = 12
NCORES = 8
P = 128            # partitions / per-core hidden slice
GS = 4 * P         # per-core gate slice (i,f,g,o each P wide) = 512
NCHUNK = 512       # node-column chunk (PSUM bank = 512 fp32)
F32 = mybir.dt.float32
BF16 = mybir.dt.bfloat16
F8 = mybir.dt.float8e4
AF = mybir.ActivationFunctionType

_CACHE = {}


def _h_dtype(k):
    """dtype of level-k h as gathered/consumed by level k-1."""
    return F8 if k >= 7 else BF16


def _build():
    nc = bacc.Bacc(
        "TRN2",
        target_bir_lowering=False,
        debug=False,
        enable_asserts=False,
        num_devices=NCORES,
    )

    # Host-packed layouts: per-partition-contiguous so each load is 128
    # descriptors of 8-16KB (startup is DMA-descriptor-throughput-bound
    # with feature-major layouts: thousands of 1KB rows).
    # embP chunks 0..3 = xw cols [0:2048), 4..7 = leaf cols [2047:4095).
    embP_d = nc.dram_tensor("embP", (8, P, I // P * NCHUNK), BF16,
                            kind="ExternalInput")
    wihP_d = nc.dram_tensor("wihP", (P, I // P * GS), BF16, kind="ExternalInput")
    whhP_d = nc.dram_tensor("whhP", (P, 2 * H // P * GS), BF16,
                            kind="ExternalInput")
    bias_d = nc.dram_tensor("bias", (P, 4), F32, kind="ExternalInput")
    ident_d = nc.dram_tensor("ident", (P, P), BF16, kind="ExternalInput")
    out_d = nc.dram_tensor("out", (2 * P, 1), F32, kind="ExternalOutput")

    KX = I // P        # 8 contraction chunks for the x part
    KH = 2 * H // P    # 16 contraction chunks for the hh part
    rg = [list(range(NCORES))]

    with tile.TileContext(nc) as tc:
        with (
            tc.tile_pool(name="wpool", bufs=1) as wpool,
            tc.tile_pool(name="xpool", bufs=2) as xpool,
            tc.tile_pool(name="spool", bufs=2) as spool,
            tc.tile_pool(name="state", bufs=2) as state,
            tc.tile_pool(name="ewpool", bufs=2) as ewpool,
            tc.tile_pool(name="psum", bufs=8, space=bass.MemorySpace.PSUM) as psum,
            tc.tile_pool(name="dram", bufs=2, space=bass.MemorySpace.DRAM) as dram,
        ):
            # ---- warmup collective: absorb the cold-start cost ----------
            wz = wpool.tile([P, 1], F32)
            nc.vector.memset(wz[:], 0.0)
            wag_in = dram.tile([P, 1], F32, name="wagin")
            wag_out = dram.tile([NCORES * P, 1], F32, name="wagout",
                                addr_space="Shared")
            nc.sync.dma_start(wag_in[:], wz[:])
            nc.gpsimd.collective_compute(
                "AllGather",
                mybir.AluOpType.bypass,
                replica_groups=rg,
                ins=[wag_in.opt()],
                outs=[wag_out.opt()],
            )

            # ---- resident weights, feature-major ------------------------
            # [:, c, q*128:(q+1)*128] is the stationary (K=128, M=128) tile
            # for contraction chunk c, gate q
            wih = wpool.tile([P, KX, GS], BF16)
            whh = wpool.tile([P, KH, GS], BF16)
            bias = wpool.tile([P, 4], F32)
            ident = wpool.tile([P, P], BF16)
            nc.sync.dma_start(bias[:], bias_d[:])
            nc.sync.dma_start(ident[:], ident_d[:])
            nc.sync.dma_start(
                wih[:], wihP_d[:].rearrange("p (a g) -> p a g", a=KX)
            )
            # whh (2MB) is not needed until k=10 (~90us in); emitted after
            # leaf chunk 1 below so it doesn't steal DMA bandwidth from the
            # leaf-critical wih/ex loads.

            # ---- phase structure ----------------------------------------
            # 1. leaf level (k=11): x-only gates, elementwise, chunked;
            #    AllGathers (fp8, 1024-col chunks) start flowing early.
            # 2. XW precompute: x@W_ih.T + b for ALL interior nodes (heap
            #    rows 0..2046) into SBUF - dense PE work that runs while
            #    the leaf AllGathers drain, and removes the x part (and
            #    the bias) from the recurrent critical path entirely.
            # 3. levels 10..0: hh-only PSUM groups + xw combine + cell.
            xw = wpool.tile([P, 4, 4 * NCHUNK], BF16)  # (128, 4, 2048)

            lvl = {}

            def get_level(k):
                if k not in lvl:
                    n = 2 ** k
                    h_new = state.tile(
                        [P, max(n, 2)], _h_dtype(k), tag="hst", bufs=2,
                        name=f"h{k}"
                    )
                    c_new = state.tile(
                        [P, max(n, 2)], F32, tag="cst", bufs=3, name=f"c{k}"
                    )
                    lvl[k] = {"h": h_new, "c": c_new, "hgat": []}
                return lvl[k]

            def emit_ag(k, p0, pw):
                """AllGather h_new[:, p0:p0+pw] to all cores (via DRAM)."""
                L = lvl[k]
                hdt = _h_dtype(k)
                ag_in = dram.tile([P, pw], hdt, tag="agin", bufs=6,
                                  name=f"agin{k}_{p0}")
                ag_out = dram.tile([NCORES * P, pw], hdt, tag="agout",
                                   bufs=10, name=f"agout{k}_{p0}",
                                   addr_space="Shared")
                nc.sync.dma_start(ag_in[:], L["h"][:, p0:p0 + pw])
                nc.gpsimd.collective_compute(
                    "AllGather",
                    mybir.AluOpType.bypass,
                    replica_groups=rg,
                    ins=[ag_in.opt()],
                    outs=[ag_out.opt()],
                )
                L["hgat"].append((ag_out, pw))

            # ---- phase 1: leaf level ------------------------------------
            K = DEPTH - 1
            nl = 2 ** K
            get_level(K)
            h_leaf, c_leaf = lvl[K]["h"], lvl[K]["c"]
            for j in range(nl // NCHUNK):
                j0 = j * NCHUNK
                ex = xpool.tile([P, KX, NCHUNK], BF16, tag="ex", name=f"exL{j}")
                eng = nc.sync if j % 2 == 0 else nc.scalar
                eng.dma_start(
                    ex[:],
                    embP_d[4 + j].rearrange("p (a w) -> p a w", a=KX),
                )
                # gate order is (g, i, f, o); f is skipped at the leaf
                ps = [None] * 4
                for q in (0, 1, 3):
                    ps[q] = psum.tile([P, NCHUNK], F32, tag="ps", name=f"psL{j}_{q}")
                for q in (0, 1, 3):
                    for a in range(KX):
                        nc.tensor.matmul(
                            ps[q][:], wih[:, a, q * P:(q + 1) * P], ex[:, a, :],
                            start=(a == 0), stop=(a == KX - 1),
                        )
                # leaf cell: c = sig(i)*tanh(g), h = sig(o)*tanh(c)
                sig_i = ewpool.tile([P, NCHUNK], F32, tag="si")
                tan_g = ewpool.tile([P, NCHUNK], F32, tag="tg")
                sig_o = ewpool.tile([P, NCHUNK], F32, tag="so")
                nc.scalar.activation(tan_g[:], ps[0][:], AF.Tanh, bias=bias[:, 0:1])
                nc.scalar.activation(sig_i[:], ps[1][:], AF.Sigmoid, bias=bias[:, 1:2])
                nc.scalar.activation(sig_o[:], ps[3][:], AF.Sigmoid, bias=bias[:, 3:4])
                nc.vector.tensor_mul(c_leaf[:, j0:j0 + NCHUNK], sig_i[:], tan_g[:])
                tan_c = ewpool.tile([P, NCHUNK], F32, tag="tc")
                nc.scalar.activation(tan_c[:], c_leaf[:, j0:j0 + NCHUNK], AF.Tanh)
                nc.vector.tensor_mul(h_leaf[:, j0:j0 + NCHUNK], sig_o[:], tan_c[:])
                if (j0 + NCHUNK) % 1024 == 0:
                    emit_ag(K, j0 + NCHUNK - 1024, 1024)
                if j == 1:
                    nc.scalar.dma_start(
                        whh[:], whhP_d[:].rearrange("p (c g) -> p c g", c=KH)
                    )

            # ---- phase 2: XW precompute for heap rows 0..2047 -----------
            for j in range(4):
                j0 = j * NCHUNK
                ex = xpool.tile([P, KX, NCHUNK], BF16, tag="ex", name=f"exP{j}")
                eng = nc.sync if j % 2 == 0 else nc.scalar
                eng.dma_start(
                    ex[:],
                    embP_d[j].rearrange("p (a w) -> p a w", a=KX),
                )
                for q in range(4):
                    pt = psum.tile([P, NCHUNK], F32, tag="ps", name=f"psP{j}_{q}")
                    for a in range(KX):
                        nc.tensor.matmul(
                            pt[:], wih[:, a, q * P:(q + 1) * P], ex[:, a, :],
                            start=(a == 0), stop=(a == KX - 1),
                        )
                    # fold the bias in here: downstream gates are biasless
                    nc.scalar.activation(
                        xw[:, q, j0:j0 + NCHUNK], pt[:], AF.Identity,
                        bias=bias[:, q:q + 1]
                    )

            # ---- phase 3: recurrent sweep, hh only ----------------------
            slab_engines = [nc.sync, nc.scalar]
            for k in range(DEPTH - 2, -1, -1):
                n = 2 ** k
                base = n - 1
                get_level(k)
                h_new, c_new = lvl[k]["h"], lvl[k]["c"]
                c_prev = lvl[k + 1]["c"]
                hgat = lvl[k + 1]["hgat"]
                sdt = _h_dtype(k + 1)
                # NB: chunking k=9 at 256 was tried and LOST ~5us: the
                # matmul phase is LDW/dispatch-bound (~0.26us per matmul
                # regardless of rows<=512), so halving rows doubles LDW.
                cw = NCHUNK if k >= 9 else max(n, 1)
                for j in range((n + cw - 1) // cw):
                    j0 = j * cw
                    w = min(cw, n - j0)
                    wp = max(w, 2)

                    # gathered h.T slab: [p, cb, col] = h[cb*128+p, col];
                    # cols = level-(k+1) node index (2 children per node).
                    # 2-way split across the HW-DGE queues (sync/scalar),
                    # 4 cb-blocks each; avoids gpsimd's ~1us SWDGE overhead
                    # and keeps gpsimd free for collective dispatch.
                    slab = spool.tile([P, KX, 2 * wp], sdt, tag="slab",
                                      name=f"sl{k}_{j}")
                    pw = hgat[0][1]
                    nsplit = len(slab_engines)
                    cbs = KX // nsplit  # cb-blocks per split
                    for i, eng in enumerate(slab_engines):
                        r0, r1 = i * cbs * P, (i + 1) * cbs * P
                        pos, off, need = 2 * j0, 0, 2 * w
                        while need > 0:
                            pj, pc = divmod(pos, pw)
                            take = min(need, pw - pc)
                            eng.dma_start(
                                slab[:, i * cbs:(i + 1) * cbs, off:off + take],
                                hgat[pj][0][r0:r1, pc:pc + take].rearrange(
                                    "(c p) w -> p c w", p=P
                                ),
                            )
                            pos += take; off += take; need -= take
                        if wp != w:
                            eng.dma_start(
                                slab[:, i * cbs:(i + 1) * cbs, 2 * w:4 * w],
                                hgat[0][0][r0:r1, 0:2 * w].rearrange(
                                    "(c p) w -> p c w", p=P
                                ),
                            )

                    fused = 4 * wp <= NCHUNK  # all 4 gates in one PSUM bank
                    if fused:
                        ps4 = psum.tile([P, 4, wp], F32, tag="ps",
                                        name=f"ps{k}_{j}")
                        outs = [ps4[:, q] for q in range(4)]
                    else:
                        tiles = [psum.tile([P, wp], F32, tag="ps",
                                           name=f"ps{k}_{j}_{q}") for q in range(4)]
                        outs = [t[:] for t in tiles]
                    # gates = xw (identity matmul seeds psum with the
                    # precomputed x part + bias) + hh
                    for q in range(4):
                        nc.tensor.matmul(
                            outs[q], ident[:],
                            xw[:, q, base + j0: base + j0 + wp],
                            start=True, stop=False,
                        )
                        for c in range(KH):
                            nc.tensor.matmul(
                                outs[q],
                                whh[:, c, q * P:(q + 1) * P],
                                slab[:, c % KX, (c // KX)::2],
                                start=False, stop=(c == KH - 1),
                            )

                    # LSTM cell (biasless); gate order (g,i,f,o): i,f,o
                    # share one fused sigmoid when they live in one bank
                    sifo = ewpool.tile([P, 3, wp], F32, tag="sifo")
                    tg = ewpool.tile([P, wp], F32, tag="tg2")
                    if fused:
                        nc.scalar.activation(sifo[:], ps4[:, 1:4], AF.Sigmoid)
                    else:
                        nc.scalar.activation(sifo[:, 0], outs[1], AF.Sigmoid)
                        nc.scalar.activation(sifo[:, 1], outs[2], AF.Sigmoid)
                        nc.scalar.activation(sifo[:, 2], outs[3], AF.Sigmoid)
                    nc.scalar.activation(tg[:], outs[0], AF.Tanh)

                    t2 = ewpool.tile([P, wp], F32, tag="t2")
                    nc.vector.tensor_mul(t2[:], sifo[:, 0], tg[:])
                    if wp == w:
                        c_left = c_prev[:, 2 * j0: 2 * j0 + 2 * w: 2]
                    else:
                        c_left = c_prev[:, 0:2]
                    t1 = ewpool.tile([P, wp], F32, tag="t1")
                    nc.vector.tensor_mul(t1[:], sifo[:, 1], c_left)
                    nc.vector.tensor_add(c_new[:, j0:j0 + wp], t1[:], t2[:])

                    tan_c = ewpool.tile([P, wp], F32, tag="tc")
                    nc.scalar.activation(tan_c[:], c_new[:, j0:j0 + wp], AF.Tanh)
                    if k > 0:
                        nc.vector.tensor_mul(h_new[:, j0:j0 + wp], sifo[:, 2], tan_c[:])
                        emit_ag(k, j0, w)
                    else:
                        h_root = ewpool.tile([P, 2], F32, tag="hroot")
                        nc.vector.tensor_mul(h_root[:], sifo[:, 2], tan_c[:])
                        nc.sync.dma_start(out_d[0:P, :], h_root[:, 0:1])
                        nc.sync.dma_start(out_d[P:2 * P, :], c_new[:, 0:1])

    nc.compile()
    return nc


def _prep_inputs(emb, W_ih, W_hh, b_ih, b_hh):
    """Host-side sharding: kept-gate rows, per-core slices, transposes."""
    import ml_dtypes

    emb = np.asarray(emb, dtype=np.float32)
    W_ih = np.asarray(W_ih, dtype=np.float32)
    W_hh = np.asarray(W_hh, dtype=np.float32)
    b = np.asarray(b_ih, dtype=np.float32) + np.asarray(b_hh, dtype=np.float32)

    embT = emb.T.astype(ml_dtypes.bfloat16)  # (I, 4095)
    # pack per-partition-contiguous: embP[ci, p, a*512+w] = embT[a*128+p, c0+w]
    embP = np.empty((8, P, 8 * NCHUNK), ml_dtypes.bfloat16)
    nl1 = 2 ** (DEPTH - 1) - 1  # 2047
    for ci in range(8):
        c0 = ci * NCHUNK if ci < 4 else nl1 + (ci - 4) * NCHUNK
        blk = embT[:, c0:c0 + NCHUNK].reshape(8, P, NCHUNK)  # (a, p, w)
        embP[ci] = blk.transpose(1, 0, 2).reshape(P, 8 * NCHUNK)
    in_maps = []
    for m in range(NCORES):
        # gate block order (g, i, f, o): i,f,o adjacent so the kernel can
        # run one fused sigmoid over all three
        rows = np.concatenate(
            [np.arange(q * 2 * H + m * P, q * 2 * H + m * P + P)
             for q in (2, 0, 1, 3)]
        )
        wihT = W_ih[rows, :].T.astype(ml_dtypes.bfloat16)   # (1024, 512)
        whhT = W_hh[rows, :].T.astype(ml_dtypes.bfloat16)   # (2048, 512)
        wihP = np.ascontiguousarray(
            wihT.reshape(8, P, GS).transpose(1, 0, 2).reshape(P, 8 * GS)
        )
        whhP = np.ascontiguousarray(
            whhT.reshape(16, P, GS).transpose(1, 0, 2).reshape(P, 16 * GS)
        )
        bias = np.ascontiguousarray(b[rows].reshape(4, P).T)  # (128, 4)
        ident = np.eye(P, dtype=np.float32).astype(ml_dtypes.bfloat16)
        in_maps.append({"embP": embP, "wihP": wihP, "whhP": whhP,
                        "bias": bias, "ident": ident})
    return in_maps


def _install_profile_hook():
    """The agent image's antenv lacks axon_hooks; synthesize it so
    run_bass_kernel_spmd(trace=True) can capture NTFF profiles."""
    import types

    if "antenv.axon_hooks" in sys.modules:
        return
    try:
        from trn_agent_boot.trn_boot import _ntff_profile_via_ctypes
    except ImportError:
        return
    hook = _ntff_profile_via_ctypes("/opt/axon/libaxon_pjrt.so")
    mod = types.ModuleType("antenv.axon_hooks")
    mod._hook = hook
    mod.set_axon_ntff_profile_hook = lambda h: setattr(mod, "_hook", h)
    mod.get_axon_ntff_profile_hook = lambda: mod._hook
    sys.modules["antenv.axon_hooks"] = mod
    import antenv

    antenv.axon_hooks = mod


def _run(in_maps, trace=False):
    if trace:
        _install_profile_hook()
    if "nc" not in _CACHE:
        _CACHE["nc"] = _build()
    nc = _CACHE["nc"]
    res = bass_utils.run_bass_kernel_spmd(
        nc, in_maps, core_ids=list(range(NCORES)), trace=trace
    )
    return res


def _assemble(results):
    out = np.zeros((1, 2 * H), dtype=np.float32)
    for m in range(NCORES):
        o = results[m]["out"].reshape(2 * P)
        out[0, m * P:(m + 1) * P] = o[0:P]
        out[0, H + m * P: H + (m + 1) * P] = o[P:2 * P]
    return out


def kernel(emb, W_ih, W_hh, b_ih, b_hh):
    in_maps = _prep_inputs(emb, W_ih, W_hh, b_ih, b_hh)
    res = _run(in_maps, trace=False)
    return _assemble(res.results)
